# revision 1
# baseline (speedup 1.0000x reference)
"""Trainium2 Bass kernel for nn_DPP: batched masked-Gram logdet minus shared
normalizer logdet.

out[i] = logdet(G * m_i m_i^T + diag(1-m_i)) - logdet(G + I),  G = B^T B

Sharding: data-parallel over the batch dim of x (one sample per NeuronCore,
B replicated). Each core computes its sample's masked logdet AND the shared
logdet(G+I) (redundantly -- no cross-core traffic); the host gathers the 8
scalars.

v3 device algorithm (per core):
  - The masked problem is COMPRESSED on host: only the selected columns of B
    (<=1075 of 2048) are shipped as bsel [2048 x 1152], so the masked
    factorization runs at 1152 instead of 2048 and needs no mask DVE work.
  - B is shipped pre-scaled (x32) and pre-cast to fp8e4m3. All Gram work is
    fp8 DoubleRow matmuls; everything downstream runs in the x1024 scaled
    space and a closed-form constant corrects the output.
  - Two interleaved left-looking blocked Choleskys (U-form, 128 panels):
    masked A1 = bsel^T bsel + diag(pad), shared A2 = bfull^T bfull + S2*I.
    Panel strips are fused PSUM chains: fp8-DR Gram matmuls (kt-outer,
    stationary reused across the strip) + bf16 accumulation matmuls against
    NEGATED U panels (so no DVE subtract pass).
  - Each panel's strip is split across two PSUM regions with independent
    rotation: a 1-bank "diag" region (first 512 cols, incl. the 128x128
    pivot) and a "rest" region. The NEXT panel's diag chain runs as PE
    filler during the CURRENT panel's pivot refine, and the current panel's
    rest chain + evacuation overlap the refine as well -- the pivot is ready
    the moment a round starts.
  - Each 128x128 diagonal pivot S is handled matmul-only ("refine"):
      d = diag(S); r = 1/sqrt(d)                  (DVE reciprocal + ACT Sqrt)
      corr = S * (r r^T); X1 = striu(corr); X1T = stril(corr)
      W = diag(r) (I - X1 + X1@X1)                (approx inv-chol factor)
      F = W^T S W - I                             (small: ||F|| ~ 0.15)
      logdet(S) = sum(ln d) + tr F - tr F^2/2 + tr F^3/3
      What = W + W(-F/2 + 3F^2/8)                 (What What^T ~ S^{-1})
    Panel: U_strip = What^T @ strip; U evacuated on Scalar and -U on Vector
    in parallel. All ln d batched into one ACT Ln at the end.
"""

import numpy as np
import ml_dtypes

P = 128
N = 2048            # shared matrix dim (= n columns of B)
NM = 1152           # masked compressed dim (max n_sel 1075 for these inputs)
NT = N // P         # 16 shared panels
MT = NM // P        # 9 masked panels
NKT = 16            # contraction tiles (B rows padded 2000 -> 2048)
FT = 512            # free-dim tile for bf16 matmuls / psum bank width (f32)
DRT = 256           # free-dim tile for fp8 DoubleRow matmuls (2*256 = 512 AP)
DW = 512            # diag-region width (one psum bank)
SCALE = 32.0        # host pre-scale of B before fp8 cast
S2 = SCALE * SCALE  # Gram scale (1024); logdets shift by dim*ln(S2)
LN_S2 = float(np.log(S2))
OUT_CONST = (N - NM) * LN_S2   # (ld_m - NM ln S2) - (ld_s - N ln S2) fix

# masked panel i is processed in round RM[i] (tail-heavy: masked chains
# provide PE filler for the late shared rounds, whose own chains are thin)
RM = [1, 3, 5, 7, 9, 11, 13, 14, 15]

_CACHE = {}


def _chunks(width, base, step, diag_first=False):
    out = []
    c = base
    end = base + width
    if diag_first:
        out.append((c, P))
        c += P
    while c < end:
        w = min(step, end - c)
        out.append((c, w))
        c += w
    return out


def _build():
    import os
    import concourse.bass as bass
    import concourse.bacc as bacc
    import concourse.mybir as mybir
    from concourse.bass import ds, ts
    from concourse.masks import (
        make_identity,
        make_upper_triangular,
        make_lower_triangular,
    )
    from concourse.tile import TileContext
    from contextlib import ExitStack

    f32 = mybir.dt.float32
    bf16 = mybir.dt.bfloat16
    fp8 = mybir.dt.float8e4
    AF = mybir.ActivationFunctionType
    OP = mybir.AluOpType
    DR = mybir.MatmulPerfMode.DoubleRow
    PSUM = bass.MemorySpace.PSUM
    AX = mybir.AxisListType.X

    dbg = os.environ.get("KDBG", "0") == "1"

    nc = bacc.Bacc()
    bb = nc.dram_tensor("bb", [N, N], fp8, kind="ExternalInput")
    bs_d = nc.dram_tensor("bs", [N, NM], fp8, kind="ExternalInput")
    pad_d = nc.dram_tensor("pad", [NM, 1], f32, kind="ExternalInput")
    out_d = nc.dram_tensor("out", [1, 1], f32, kind="ExternalOutput")
    if dbg:
        dst_d = nc.dram_tensor("dst", [P, 2 * NT], f32, kind="ExternalOutput")

    TDIM = [MT, NT]  # panels per matrix (m=0 masked, m=1 shared)
    DIMW = [NM, N]

    with TileContext(nc) as tc, ExitStack() as stack:
        consts = stack.enter_context(tc.tile_pool(name="consts", bufs=1))
        I128 = consts.tile([P, P], f32, tag="i128")
        make_identity(nc, I128)
        I128b = consts.tile([P, P], bf16, tag="i128b")
        nc.vector.tensor_copy(I128b, I128)
        STRIU = consts.tile([P, P], f32, tag="striu")
        make_upper_triangular(nc, STRIU, val=1.0, diag=False)
        STRIL = consts.tile([P, P], f32, tag="stril")
        make_lower_triangular(nc, STRIL, val=1.0, diag=False)

        padc = consts.tile([P, MT], f32, tag="padc")
        nc.sync.dma_start(padc, pad_d.rearrange("(t p) one -> p (t one)", p=P))
        acc = consts.tile([P, 2], f32, tag="acc")
        nc.vector.memset(acc, 0.0)
        dstore = consts.tile([P, 2, NT], f32, tag="dstore")
        nc.vector.memset(dstore, 1.0)  # unused masked cols -> ln 1 = 0
        # diag fixes (in scaled space): masked pad cols get S2 on the diag,
        # shared panels get S2*I
        dfix_m = consts.tile([P, MT, P], f32, tag="dfix_m")
        pscl = consts.tile([P, MT], f32, tag="pscl")
        nc.vector.tensor_scalar(
            out=pscl, in0=padc, scalar1=S2, scalar2=None, op0=OP.mult
        )
        for i in range(MT):
            nc.vector.tensor_scalar_mul(dfix_m[:, i, :], I128, pscl[:, ds(i, 1)])
        dfix_s = consts.tile([P, P], f32, tag="dfix_s")
        nc.vector.tensor_scalar(
            out=dfix_s, in0=I128, scalar1=S2, scalar2=None, op0=OP.mult
        )

        # fp8 inputs, [p, kt, cols]; DMAs are emitted interleaved with the
        # panel-0 chains below
        bsel = consts.tile([P, NKT, NM], fp8, tag="bsel")
        bful = consts.tile([P, NKT, N], fp8, tag="bful")
        bs_r = bs_d.rearrange("(t p) n -> p t n", p=P)
        bb_r = bb.rearrange("(t p) n -> p t n", p=P)

        # U panels (bf16) and negated copies (stationary side of -U^T U)
        ub = {}
        un = {}
        for m in range(2):
            for i in range(TDIM[m]):
                w = (TDIM[m] - i) * P
                ub[(m, i)] = consts.tile(
                    [P, w], bf16, tag=f"ub{m}_{i}", name=f"ub{m}_{i}"
                )
                if i < TDIM[m] - 1:
                    un[(m, i)] = consts.tile(
                        [P, w - P], bf16, tag=f"un{m}_{i}", name=f"un{m}_{i}"
                    )

        # PSUM (8 banks): per-matrix diag bank (1+1) + rest region (2+3) +
        # one work bank for refine/TRSM rotations
        mdiag = stack.enter_context(tc.tile_pool(name="mdiag", bufs=1, space=PSUM))
        sdiag = stack.enter_context(tc.tile_pool(name="sdiag", bufs=1, space=PSUM))
        mrest = stack.enter_context(tc.tile_pool(name="mrest", bufs=1, space=PSUM))
        srest = stack.enter_context(tc.tile_pool(name="srest", bufs=1, space=PSUM))
        wpsum = stack.enter_context(tc.tile_pool(name="wpsum", bufs=1, space=PSUM))
        spool = stack.enter_context(tc.tile_pool(name="spool", bufs=2))
        rpool = stack.enter_context(tc.tile_pool(name="rpool", bufs=2))
        vpool = stack.enter_context(tc.tile_pool(name="vpool", bufs=2))

        X = [bsel, bful]
        DPOOL = [mdiag, sdiag]
        RPOOL = [mrest, srest]
        RESTW = [NM - DW, N - DW]

        def new_panel(m, i):
            T = TDIM[m]
            w = (T - i) * P
            cx = {
                "w": w,
                "dw": min(DW, w),
                "dp": DPOOL[m].tile([P, DW], f32, tag=f"dp{m}", name="dp"),
                "rp": None,
                "sblk": rpool.tile([P, P], f32, tag="sblk", name="sblk"),
                "sb": rpool.tile([P, P], bf16, tag="sb", name="sb"),
                "strip": None,
                "jmax": 0,
            }
            if w > DW:
                cx["rp"] = RPOOL[m].tile(
                    [P, RESTW[m]], f32, tag=f"rp{m}", name="rp"
                )
            if w > P:
                cx["strip"] = spool.tile(
                    [P, w - P], bf16, tag=f"strip{m}", name="strip"
                )
            return cx

        def diag_chain(m, i, cx, jmax, dma=None):
            """Gram (fp8 DR, kt-outer) + accum j<jmax for strip cols
            [c0, c0+dw) into the 1-bank diag psum. Yields per matmul."""
            dp, dw = cx["dp"], cx["dw"]
            c0 = i * P
            cx["jmax"] = jmax
            for kp in range(NKT // 2):
                if dma is not None:
                    dma(kp)
                stat = X[m][:, 2 * kp : 2 * kp + 2, ds(c0, P)]
                for (cc, cw) in _chunks(dw, c0, DRT):
                    nc.tensor.matmul(
                        dp[:, ds(cc - c0, cw)],
                        stat,
                        X[m][:, 2 * kp : 2 * kp + 2, ds(cc, cw)],
                        start=(kp == 0 and cc == c0),
                        stop=False,
                        perf_mode=DR,
                        skip_group_check=True,
                    )
                    yield
            for j in range(jmax):
                rel = (i - j) * P
                nc.tensor.matmul(
                    dp[:, :dw],
                    un[(m, j)][:, ds(rel - P, P)],
                    ub[(m, j)][:, ds(rel, dw)],
                    start=False,
                    stop=False,
                    skip_group_check=True,
                )
                yield

        def diag_accum_tail(m, i, cx, jhi):
            """Late accum terms j in [jmax, jhi) into the diag psum (U_j only
            became available after the chain was queued)."""
            dp, dw = cx["dp"], cx["dw"]
            for j in range(cx["jmax"], jhi):
                rel = (i - j) * P
                nc.tensor.matmul(
                    dp[:, :dw],
                    un[(m, j)][:, ds(rel - P, P)],
                    ub[(m, j)][:, ds(rel, dw)],
                    start=False,
                    stop=True,
                    skip_group_check=True,
                )
            cx["jmax"] = jhi

        def rest_chain(m, i, cx):
            """Gram + full accum for strip cols [c0+DW, c0+w) into the rest
            psum, then evacuate those cols to the SBUF strip (scalar).
            Runs entirely as filler during this panel's refine."""
            w, rp = cx["w"], cx["rp"]
            if rp is None:
                return
            c0 = i * P
            rw = w - DW
            for kp in range(NKT // 2):
                stat = X[m][:, 2 * kp : 2 * kp + 2, ds(c0, P)]
                for (cc, cw) in _chunks(rw, c0 + DW, DRT):
                    rel = cc - c0 - DW
                    nc.tensor.matmul(
                        rp[:, ds(rel, cw)],
                        stat,
                        X[m][:, 2 * kp : 2 * kp + 2, ds(cc, cw)],
                        start=(kp == 0 and rel % FT == 0),
                        stop=False,
                        perf_mode=DR,
                        skip_group_check=True,
                    )
                    yield
            for j in range(i):
                relj = (i - j) * P
                for (cc, cw) in _chunks(rw, c0 + DW, FT):
                    nc.tensor.matmul(
                        rp[:, ds(cc - c0 - DW, cw)],
                        un[(m, j)][:, ds(relj - P, P)],
                        ub[(m, j)][:, ds(relj + cc - c0, cw)],
                        start=False,
                        stop=(j == i - 1),
                        skip_group_check=True,
                    )
                    yield
            # evacuate rest cols to the SBUF strip (scalar; chunked so the
            # TRSM and the next rest rotation unblock per-chunk)
            strip = cx["strip"]
            for (cc, cw) in _chunks(rw, 0, FT):
                nc.scalar.copy(
                    strip[:, ds(DW - P + cc, cw)], rp[:, ds(cc, cw)]
                )
                yield

        def evac_pivot(m, i, cx):
            """Pivot -> sblk (+fix) on DVE. dp is complete at round start."""
            dp = cx["dp"]
            dfix = dfix_m[:, i, :] if m == 0 else dfix_s
            nc.vector.tensor_add(cx["sblk"], dp[:, :P], dfix)
            nc.vector.tensor_copy(cx["sb"], cx["sblk"])

        def evac_dstrip(m, i, cx):
            """Diag-region cols [P, dw) -> SBUF strip (scalar). Emitted after
            the refines' first section so sqrt isn't queued behind it."""
            dp, w, dw = cx["dp"], cx["w"], cx["dw"]
            if w > P:
                nc.scalar.copy(cx["strip"][:, : dw - P], dp[:, ds(P, dw - P)])

        def refine_gen(m, i, cx):
            """Pivot-block factor; yields at cross-engine handoffs."""
            sblk, sb = cx["sblk"], cx["sb"]
            dcol = dstore[:, m, ds(i, 1)]
            dummy = rpool.tile([P, P], f32, tag="dummy", name="dummy")
            nc.vector.tensor_mul(dummy, sblk, I128)
            nc.vector.tensor_reduce(dcol, dummy, AX, OP.add)
            rinv = vpool.tile([P, 1], f32, tag="rinv", name="rinv")
            nc.vector.reciprocal(rinv, dcol)
            rcol = vpool.tile([P, 1], f32, tag="rcol", name="rcol")
            nc.scalar.sqrt(rcol, rinv)
            yield
            rt_ps = wpsum.tile([P, FT], f32, tag="w", name="rt_ps")
            nc.tensor.transpose(rt_ps[:1, :P], rcol, I128)
            rrow = vpool.tile([1, P], bf16, tag="rrow", name="rrow")
            nc.vector.tensor_copy(rrow, rt_ps[:1, :P])
            yield
            q_ps = wpsum.tile([P, FT], f32, tag="w", name="q_ps")
            nc.tensor.matmul(q_ps[:, :P], rrow, rrow, start=True, stop=True)
            c1 = rpool.tile([P, P], f32, tag="c1", name="c1")
            nc.vector.tensor_mul(c1, sblk, q_ps[:, :P])
            yield
            x1 = rpool.tile([P, P], bf16, tag="x1", name="x1")
            nc.gpsimd.tensor_mul(x1, c1, STRIU)
            x1t = rpool.tile([P, P], bf16, tag="x1t", name="x1t")
            nc.gpsimd.tensor_mul(x1t, c1, STRIL)
            yield
            x2_ps = wpsum.tile([P, FT], f32, tag="w", name="x2_ps")
            nc.tensor.matmul(x2_ps[:, :P], x1t, x1, start=True, stop=True)
            wser = rpool.tile([P, P], f32, tag="wser", name="wser")
            nc.vector.tensor_sub(wser, x2_ps[:, :P], x1)
            nc.vector.tensor_add(wser, wser, I128)
            wfac = rpool.tile([P, P], bf16, tag="wfac", name="wfac")
            nc.vector.tensor_scalar_mul(wfac, wser, rcol)
            yield
            wt_ps = wpsum.tile([P, FT * 2], bf16, tag="w", name="wt_ps")
            nc.tensor.transpose(wt_ps[:, :P], wfac, I128b)
            wt = rpool.tile([P, P], bf16, tag="wt", name="wt")
            nc.vector.tensor_copy(wt, wt_ps[:, :P])
            yield
            sw_ps = wpsum.tile([P, FT], f32, tag="w", name="sw_ps")
            nc.tensor.matmul(sw_ps[:, :P], sb, wfac, start=True, stop=True)
            swt = rpool.tile([P, P], bf16, tag="swt", name="swt")
            nc.vector.tensor_copy(swt, sw_ps[:, :P])
            yield
            fpi_ps = wpsum.tile([P, FT], f32, tag="w", name="fpi_ps")
            nc.tensor.matmul(fpi_ps[:, :P], wfac, swt, start=True, stop=True)
            ff = rpool.tile([P, P], bf16, tag="ff", name="ff")
            nc.vector.tensor_sub(ff, fpi_ps[:, :P], I128)
            trf = vpool.tile([P, 1], f32, tag="trf", name="trf")
            dummy3 = rpool.tile([P, P], f32, tag="dummy3", name="dummy3")
            nc.gpsimd.tensor_mul(dummy3, ff, I128)
            nc.vector.tensor_reduce(trf, dummy3, AX, OP.add)
            trf2 = vpool.tile([P, 1], f32, tag="trf2", name="trf2")
            dummy4 = rpool.tile([P, P], f32, tag="dummy4", name="dummy4")
            nc.gpsimd.tensor_mul(dummy4, ff, ff)
            nc.vector.tensor_reduce(trf2, dummy4, AX, OP.add)
            yield
            f2_ps = wpsum.tile([P, FT], f32, tag="w", name="f2_ps")
            nc.tensor.matmul(f2_ps[:, :P], ff, ff, start=True, stop=True)
            trf3 = vpool.tile([P, 1], f32, tag="trf3", name="trf3")
            dummy5 = rpool.tile([P, P], f32, tag="dummy5", name="dummy5")
            nc.vector.tensor_mul(dummy5, f2_ps[:, :P], ff)
            nc.vector.tensor_reduce(trf3, dummy5, AX, OP.add)
            f2s = rpool.tile([P, P], bf16, tag="f2s", name="f2s")
            nc.vector.tensor_scalar_mul(f2s, f2_ps[:, :P], 0.375)
            fs = rpool.tile([P, P], bf16, tag="fs", name="fs")
            nc.vector.tensor_scalar_mul(fs, ff, -0.5)
            yield
            wh_ps = wpsum.tile([P, FT], f32, tag="w", name="wh_ps")
            nc.tensor.matmul(wh_ps[:, :P], wt, fs, start=True, stop=False)
            nc.tensor.matmul(wh_ps[:, :P], wt, f2s, start=False, stop=True)
            what = rpool.tile([P, P], bf16, tag="what", name="what")
            nc.vector.tensor_add(what, wh_ps[:, :P], wfac)
            cx["what"] = what
            # logdet trace series accumulation
            t1 = vpool.tile([P, 1], f32, tag="t1", name="t1")
            t2 = vpool.tile([P, 1], f32, tag="t2", name="t2")
            nc.vector.tensor_scalar(
                out=t2, in0=trf2, scalar1=-0.5, scalar2=None, op0=OP.mult
            )
            nc.vector.tensor_add(t1, trf, t2)
            nc.vector.tensor_scalar(
                out=t2, in0=trf3, scalar1=1.0 / 3.0, scalar2=None, op0=OP.mult
            )
            nc.vector.tensor_add(t1, t1, t2)
            nc.vector.tensor_add(acc[:, ds(m, 1)], acc[:, ds(m, 1)], t1)

        def trsm_gen(m, i, cx):
            """U_i = What^T @ strip. Outputs land in the (now free) rest-psum
            banks of this panel plus <=2 work-bank rotations, so the chunk
            matmuls run back-to-back; U (scalar) and -U (vector) evacuations
            pipeline off the PE critical path."""
            T = TDIM[m]
            w = cx["w"]
            c0 = i * P
            last = i == T - 1
            tpr = None
            if w > DW:
                tpr = RPOOL[m].tile([P, RESTW[m]], f32, tag=f"rp{m}", name="tpr")
            rp_off = 0
            for tix, (cc, cw) in enumerate(_chunks(w, c0, FT, diag_first=True)):
                rhs = cx["sb"] if tix == 0 else cx["strip"][:, ds(cc - c0 - P, cw)]
                if tix > 0 and tpr is not None and rp_off + cw <= RESTW[m]:
                    tp = tpr[:, ds(rp_off, cw)]
                    rp_off += FT
                else:
                    tpw = wpsum.tile([P, FT], f32, tag="w", name="tpw")
                    tp = tpw[:, :cw]
                nc.tensor.matmul(tp, cx["what"], rhs, start=True, stop=True)
                nc.scalar.copy(ub[(m, i)][:, ds(cc - c0, cw)], tp)
                if not last and tix > 0:
                    nc.vector.tensor_scalar(
                        out=un[(m, i)][:, ds(cc - c0 - P, cw)],
                        in0=tp,
                        scalar1=-1.0,
                        scalar2=None,
                        op0=OP.mult,
                    )
                yield

        # ---------------- schedule ----------------
        rm_of_round = {r: i for i, r in enumerate(RM)}

        fillers = []  # [key, gen] providing PE filler chunks

        def pump_fillers(k=1, only=None):
            done = 0
            idx = 0
            while idx < len(fillers) and done < k:
                key, g = fillers[idx]
                if only is not None and key not in only:
                    idx += 1
                    continue
                try:
                    next(g)
                    done += 1
                except StopIteration:
                    fillers.pop(idx)

        def dma_bful(kp):
            nc.sync.dma_start(bful[:, 2 * kp, :], bb_r[:, 2 * kp, :])
            nc.sync.dma_start(bful[:, 2 * kp + 1, :], bb_r[:, 2 * kp + 1, :])

        def dma_bsel(kp):
            nc.sync.dma_start(bsel[:, 2 * kp, :], bs_r[:, 2 * kp, :])
            nc.sync.dma_start(bsel[:, 2 * kp + 1, :], bs_r[:, 2 * kp + 1, :])

        cur = {}  # m -> (i, cx) of the panel whose chains are queued/running

        # shared panel 0: diag chain eagerly (streams against its DMAs);
        # the masked panel-0 diag chain becomes round-0 filler. Rest chains
        # are queued at each panel's own processing round (their U_j accum
        # inputs all exist by then).
        cs = new_panel(1, 0)
        for _ in diag_chain(1, 0, cs, 0, dma=dma_bful):
            pass
        cur[1] = (0, cs)
        cm = new_panel(0, 0)
        fillers.append([(0, 0, "d"), diag_chain(0, 0, cm, 0, dma=dma_bsel)])
        cur[0] = (0, cm)

        for r in range(NT):
            work = []  # (m, i, cx) panels processed this round
            mi = rm_of_round.get(r)
            if mi is not None:
                work.append((0, mi, cur[0][1]))
            work.append((1, r, cur[1][1]))

            # diag psums of this round's panels are complete (chains drained
            # and tail accums emitted at the end of the previous round)
            for (m, i, cx) in work:
                evac_pivot(m, i, cx)

            # start the refines' first section (ends at the scalar sqrt)
            # before anything else lands on the scalar queue
            gens = [refine_gen(m, i, cx) for (m, i, cx) in work]
            live = list(gens)
            for g in live:
                next(g)

            # diag-region strip copies (scalar), rest chains (filler), and
            # next-panel diag chains
            nxt = {}
            for (m, i, cx) in work:
                evac_dstrip(m, i, cx)
                fillers.append([(m, i, "r"), rest_chain(m, i, cx)])
            for (m, i, cx) in work:
                if i + 1 < TDIM[m]:
                    nx = new_panel(m, i + 1)
                    fillers.append(
                        [(m, i + 1, "d"), diag_chain(m, i + 1, nx, i)]
                    )
                    nxt[m] = (i + 1, nx)
                    cur[m] = (i + 1, nx)

            # rest of the refines, with rest/diag chains as PE filler
            while live:
                for g in list(live):
                    try:
                        next(g)
                    except StopIteration:
                        live.remove(g)
                    pump_fillers(4)

            # this round's rest chains (incl. strip evac) must be done
            # before their TRSMs read the strip
            pump_fillers(100000, only={(m, i, "r") for (m, i, _) in work})

            # TRSMs, zipped with filler
            tgens = [trsm_gen(m, i, cx) for (m, i, cx) in work]
            live = list(tgens)
            while live:
                for g in list(live):
                    try:
                        next(g)
                    except StopIteration:
                        live.remove(g)
                    pump_fillers(3)

            # close out the next panels' diag psums: drain their diag chains
            # (gram + old accums), then append the just-produced U_i term
            for (m, i, cx) in work:
                if m in nxt:
                    ni, nx = nxt[m]
                    pump_fillers(100000, only={(m, ni, "d")})
                    diag_accum_tail(m, ni, nx, ni)
            # note: masked next-panel rest chains keep pumping in later
            # rounds' refine gaps; they are force-drained before their TRSM
        pump_fillers(100000)  # drain any tail

        # -------- final: batched Ln(d), partition-sum via matmul ------
        lnall = vpool.tile([P, 2, NT], f32, tag="lnall", name="lnall")
        nc.scalar.activation(
            lnall.rearrange("p a b -> p (a b)"),
            dstore.rearrange("p a b -> p (a b)"),
            AF.Ln,
        )
        ln0 = vpool.tile([P, 1], f32, tag="ln0", name="ln0")
        nc.vector.tensor_reduce(ln0, lnall[:, 0, :], AX, OP.add)
        ln1 = vpool.tile([P, 1], f32, tag="ln1", name="ln1")
        nc.vector.tensor_reduce(ln1, lnall[:, 1, :], AX, OP.add)
        accd = vpool.tile([P, 1], f32, tag="accd", name="accd")
        nc.vector.tensor_sub(accd, acc[:, 0:1], acc[:, 1:2])
        nc.vector.tensor_add(accd, accd, ln0)
        nc.vector.tensor_sub(accd, accd, ln1)
        ones = vpool.tile([P, 1], f32, tag="ones", name="ones")
        nc.vector.memset(ones, 1.0)
        r_ps = wpsum.tile([P, FT], f32, tag="w", name="r_ps")
        nc.tensor.matmul(r_ps[:1, :1], accd, ones, start=True, stop=True)
        res = vpool.tile([1, 1], f32, tag="res", name="res")
        nc.vector.tensor_scalar(
            out=res, in0=r_ps[:1, :1], scalar1=1.0, scalar2=OUT_CONST,
            op0=OP.mult, op1=OP.add,
        )
        nc.sync.dma_start(out_d[:, :], res)
        if dbg:
            nc.sync.dma_start(dst_d[:, :], dstore.rearrange("p a b -> p (a b)"))

    nc.finalize()
    return nc


def make_in_maps(x, B):
    """Host-side shard/pack: per-core fp8 inputs."""
    bs, n = x.shape
    k = B.shape[0]
    b8 = np.zeros((N, N), dtype=ml_dtypes.float8_e4m3)
    b8[:k, :] = (B * SCALE).astype(ml_dtypes.float8_e4m3)
    in_maps = []
    for c in range(bs):
        sel = np.nonzero(x[c] == 1)[0]
        ns = len(sel)
        assert ns <= NM, f"sample {c} selects {ns} > {NM} columns"
        bsel = np.zeros((N, NM), dtype=ml_dtypes.float8_e4m3)
        bsel[:, :ns] = b8[:, sel]
        pad = np.zeros((NM, 1), dtype=np.float32)
        pad[ns:] = 1.0
        in_maps.append({"bb": b8, "bs": bsel, "pad": pad})
    return in_maps


def kernel(x, B):
    """Full inputs -> full output. x: [8, 2048] int32, B: [2000, 2048] f32."""
    from concourse.bass_utils import run_bass_kernel_spmd

    bs, n = x.shape
    assert n == N and bs == 8

    if "nc" not in _CACHE:
        _CACHE["nc"] = _build()
    nc = _CACHE["nc"]

    in_maps = make_in_maps(x, B)
    res = run_bass_kernel_spmd(nc, in_maps, core_ids=list(range(bs)))
    out = np.array([r["out"][0, 0] for r in res.results], dtype=np.float32)
    return out



# revision 2
# speedup vs baseline: 1.0081x; 1.0081x over previous
"""Trainium2 Bass kernel for nn_DPP: batched masked-Gram logdet minus shared
normalizer logdet.

out[i] = logdet(G * m_i m_i^T + diag(1-m_i)) - logdet(G + I),  G = B^T B

Sharding: data-parallel over the batch dim of x (one sample per NeuronCore,
B replicated). Each core computes its sample's masked logdet AND the shared
logdet(G+I) (redundantly -- no cross-core traffic); the host gathers the 8
scalars.

v3 device algorithm (per core):
  - The masked problem is COMPRESSED on host: only the selected columns of B
    (<=1075 of 2048) are shipped as bsel [2048 x 1152], so the masked
    factorization runs at 1152 instead of 2048 and needs no mask DVE work.
  - B is shipped pre-scaled (x32) and pre-cast to fp8e4m3. All Gram work is
    fp8 DoubleRow matmuls; everything downstream runs in the x1024 scaled
    space and a closed-form constant corrects the output.
  - Two interleaved left-looking blocked Choleskys (U-form, 128 panels):
    masked A1 = bsel^T bsel + diag(pad), shared A2 = bfull^T bfull + S2*I.
    Panel strips are fused PSUM chains: fp8-DR Gram matmuls (kt-outer,
    stationary reused across the strip) + bf16 accumulation matmuls against
    NEGATED U panels (so no DVE subtract pass).
  - Each panel's strip is split across two PSUM regions with independent
    rotation: a 1-bank "diag" region (first 512 cols, incl. the 128x128
    pivot) and a "rest" region. The NEXT panel's diag chain runs as PE
    filler during the CURRENT panel's pivot refine, and the current panel's
    rest chain + evacuation overlap the refine as well -- the pivot is ready
    the moment a round starts.
  - Each 128x128 diagonal pivot S is handled matmul-only ("refine"):
      d = diag(S); r = 1/sqrt(d)                  (DVE reciprocal + ACT Sqrt)
      corr = S * (r r^T); X1 = striu(corr); X1T = stril(corr)
      W = diag(r) (I - X1 + X1@X1)                (approx inv-chol factor)
      F = W^T S W - I                             (small: ||F|| ~ 0.15)
      logdet(S) = sum(ln d) + tr F - tr F^2/2 + tr F^3/3
      What = W + W(-F/2 + 3F^2/8)                 (What What^T ~ S^{-1})
    Panel: U_strip = What^T @ strip; U evacuated on Scalar and -U on Vector
    in parallel. All ln d batched into one ACT Ln at the end.
"""

import numpy as np
import ml_dtypes

P = 128
N = 2048            # shared matrix dim (= n columns of B)
NM = 1152           # masked compressed dim (max n_sel 1075 for these inputs)
NT = N // P         # 16 shared panels
MT = NM // P        # 9 masked panels
NKT = 16            # contraction tiles (B rows padded 2000 -> 2048)
FT = 512            # free-dim tile for bf16 matmuls / psum bank width (f32)
DRT = 512           # free-dim tile for fp8 DoubleRow matmuls (2*512 = 1024 AP)
DW = 512            # diag-region width (one psum bank)
SCALE = 32.0        # host pre-scale of B before fp8 cast
S2 = SCALE * SCALE  # Gram scale (1024); logdets shift by dim*ln(S2)
LN_S2 = float(np.log(S2))
OUT_CONST = (N - NM) * LN_S2   # (ld_m - NM ln S2) - (ld_s - N ln S2) fix

# masked panel i is processed in round RM[i] (tail-heavy: masked chains
# provide PE filler for the late shared rounds, whose own chains are thin)
RM = [1, 3, 5, 7, 9, 11, 13, 14, 15]

_CACHE = {}


def _chunks(width, base, step, diag_first=False):
    out = []
    c = base
    end = base + width
    if diag_first:
        out.append((c, P))
        c += P
    while c < end:
        w = min(step, end - c)
        out.append((c, w))
        c += w
    return out


def _build():
    import os
    import concourse.bass as bass
    import concourse.bacc as bacc
    import concourse.mybir as mybir
    from concourse.bass import ds, ts
    from concourse.masks import (
        make_identity,
        make_upper_triangular,
        make_lower_triangular,
    )
    from concourse.tile import TileContext
    from contextlib import ExitStack

    f32 = mybir.dt.float32
    bf16 = mybir.dt.bfloat16
    fp8 = mybir.dt.float8e4
    AF = mybir.ActivationFunctionType
    OP = mybir.AluOpType
    DR = mybir.MatmulPerfMode.DoubleRow
    PSUM = bass.MemorySpace.PSUM
    AX = mybir.AxisListType.X

    dbg = os.environ.get("KDBG", "0") == "1"

    nc = bacc.Bacc()
    bb = nc.dram_tensor("bb", [N, N], fp8, kind="ExternalInput")
    bs_d = nc.dram_tensor("bs", [N, NM], fp8, kind="ExternalInput")
    pad_d = nc.dram_tensor("pad", [NM, 1], f32, kind="ExternalInput")
    out_d = nc.dram_tensor("out", [1, 1], f32, kind="ExternalOutput")
    if dbg:
        dst_d = nc.dram_tensor("dst", [P, 2 * NT], f32, kind="ExternalOutput")

    TDIM = [MT, NT]  # panels per matrix (m=0 masked, m=1 shared)
    DIMW = [NM, N]

    with TileContext(nc) as tc, ExitStack() as stack:
        consts = stack.enter_context(tc.tile_pool(name="consts", bufs=1))
        I128 = consts.tile([P, P], f32, tag="i128")
        make_identity(nc, I128)
        I128b = consts.tile([P, P], bf16, tag="i128b")
        nc.vector.tensor_copy(I128b, I128)
        STRIU = consts.tile([P, P], f32, tag="striu")
        make_upper_triangular(nc, STRIU, val=1.0, diag=False)
        STRIL = consts.tile([P, P], f32, tag="stril")
        make_lower_triangular(nc, STRIL, val=1.0, diag=False)

        padc = consts.tile([P, MT], f32, tag="padc")
        nc.sync.dma_start(padc, pad_d.rearrange("(t p) one -> p (t one)", p=P))
        acc = consts.tile([P, 2], f32, tag="acc")
        nc.vector.memset(acc, 0.0)
        dstore = consts.tile([P, 2, NT], f32, tag="dstore")
        nc.vector.memset(dstore, 1.0)  # unused masked cols -> ln 1 = 0
        # diag fixes (in scaled space): masked pad cols get S2 on the diag,
        # shared panels get S2*I
        dfix_m = consts.tile([P, MT, P], f32, tag="dfix_m")
        pscl = consts.tile([P, MT], f32, tag="pscl")
        nc.vector.tensor_scalar(
            out=pscl, in0=padc, scalar1=S2, scalar2=None, op0=OP.mult
        )
        for i in range(MT):
            nc.vector.tensor_scalar_mul(dfix_m[:, i, :], I128, pscl[:, ds(i, 1)])
        dfix_s = consts.tile([P, P], f32, tag="dfix_s")
        nc.vector.tensor_scalar(
            out=dfix_s, in0=I128, scalar1=S2, scalar2=None, op0=OP.mult
        )

        # fp8 inputs, [p, kt, cols]; DMAs are emitted interleaved with the
        # panel-0 chains below
        bsel = consts.tile([P, NKT, NM], fp8, tag="bsel")
        bful = consts.tile([P, NKT, N], fp8, tag="bful")
        bs_r = bs_d.rearrange("(t p) n -> p t n", p=P)
        bb_r = bb.rearrange("(t p) n -> p t n", p=P)

        # U panels (bf16) and negated copies (stationary side of -U^T U)
        ub = {}
        un = {}
        for m in range(2):
            for i in range(TDIM[m]):
                w = (TDIM[m] - i) * P
                ub[(m, i)] = consts.tile(
                    [P, w], bf16, tag=f"ub{m}_{i}", name=f"ub{m}_{i}"
                )
                if i < TDIM[m] - 1:
                    un[(m, i)] = consts.tile(
                        [P, w - P], bf16, tag=f"un{m}_{i}", name=f"un{m}_{i}"
                    )

        # PSUM (8 banks): per-matrix diag bank (1+1) + rest region (2+3) +
        # one work bank for refine/TRSM rotations
        mdiag = stack.enter_context(tc.tile_pool(name="mdiag", bufs=1, space=PSUM))
        sdiag = stack.enter_context(tc.tile_pool(name="sdiag", bufs=1, space=PSUM))
        mrest = stack.enter_context(tc.tile_pool(name="mrest", bufs=1, space=PSUM))
        srest = stack.enter_context(tc.tile_pool(name="srest", bufs=1, space=PSUM))
        wpsum = stack.enter_context(tc.tile_pool(name="wpsum", bufs=1, space=PSUM))
        spool = stack.enter_context(tc.tile_pool(name="spool", bufs=2))
        rpool = stack.enter_context(tc.tile_pool(name="rpool", bufs=2))
        vpool = stack.enter_context(tc.tile_pool(name="vpool", bufs=2))

        X = [bsel, bful]
        DPOOL = [mdiag, sdiag]
        RPOOL = [mrest, srest]
        RESTW = [NM - DW, N - DW]

        def new_panel(m, i):
            T = TDIM[m]
            w = (T - i) * P
            cx = {
                "w": w,
                "dw": min(DW, w),
                "dp": DPOOL[m].tile([P, DW], f32, tag=f"dp{m}", name="dp"),
                "rp": None,
                "sblk": rpool.tile([P, P], f32, tag="sblk", name="sblk"),
                "sb": rpool.tile([P, P], bf16, tag="sb", name="sb"),
                "strip": None,
                "jmax": 0,
            }
            if w > DW:
                cx["rp"] = RPOOL[m].tile(
                    [P, RESTW[m]], f32, tag=f"rp{m}", name="rp"
                )
            if w > P:
                cx["strip"] = spool.tile(
                    [P, w - P], bf16, tag=f"strip{m}", name="strip"
                )
            return cx

        def diag_chain(m, i, cx, jmax, dma=None):
            """Gram (fp8 DR, kt-outer) + accum j<jmax for strip cols
            [c0, c0+dw) into the 1-bank diag psum. Yields per matmul."""
            dp, dw = cx["dp"], cx["dw"]
            c0 = i * P
            cx["jmax"] = jmax
            for kp in range(NKT // 2):
                if dma is not None:
                    dma(kp)
                stat = X[m][:, 2 * kp : 2 * kp + 2, ds(c0, P)]
                for (cc, cw) in _chunks(dw, c0, DRT):
                    nc.tensor.matmul(
                        dp[:, ds(cc - c0, cw)],
                        stat,
                        X[m][:, 2 * kp : 2 * kp + 2, ds(cc, cw)],
                        start=(kp == 0 and cc == c0),
                        stop=False,
                        perf_mode=DR,
                        skip_group_check=True,
                    )
                    yield
            for j in range(jmax):
                rel = (i - j) * P
                nc.tensor.matmul(
                    dp[:, :dw],
                    un[(m, j)][:, ds(rel - P, P)],
                    ub[(m, j)][:, ds(rel, dw)],
                    start=False,
                    stop=False,
                    skip_group_check=True,
                )
                yield

        def diag_accum_tail(m, i, cx, jhi):
            """Late accum terms j in [jmax, jhi) into the diag psum (U_j only
            became available after the chain was queued)."""
            dp, dw = cx["dp"], cx["dw"]
            for j in range(cx["jmax"], jhi):
                rel = (i - j) * P
                nc.tensor.matmul(
                    dp[:, :dw],
                    un[(m, j)][:, ds(rel - P, P)],
                    ub[(m, j)][:, ds(rel, dw)],
                    start=False,
                    stop=True,
                    skip_group_check=True,
                )
            cx["jmax"] = jhi

        def rest_chain(m, i, cx):
            """Gram + full accum for strip cols [c0+DW, c0+w) into the rest
            psum, then evacuate those cols to the SBUF strip (scalar).
            Runs entirely as filler during this panel's refine."""
            w, rp = cx["w"], cx["rp"]
            if rp is None:
                return
            c0 = i * P
            rw = w - DW
            for kp in range(NKT // 2):
                stat = X[m][:, 2 * kp : 2 * kp + 2, ds(c0, P)]
                for (cc, cw) in _chunks(rw, c0 + DW, DRT):
                    rel = cc - c0 - DW
                    nc.tensor.matmul(
                        rp[:, ds(rel, cw)],
                        stat,
                        X[m][:, 2 * kp : 2 * kp + 2, ds(cc, cw)],
                        start=(kp == 0 and rel % FT == 0),
                        stop=False,
                        perf_mode=DR,
                        skip_group_check=True,
                    )
                    yield
            for j in range(i):
                relj = (i - j) * P
                for (cc, cw) in _chunks(rw, c0 + DW, FT):
                    nc.tensor.matmul(
                        rp[:, ds(cc - c0 - DW, cw)],
                        un[(m, j)][:, ds(relj - P, P)],
                        ub[(m, j)][:, ds(relj + cc - c0, cw)],
                        start=False,
                        stop=(j == i - 1),
                        skip_group_check=True,
                    )
                    yield
            # evacuate rest cols to the SBUF strip (scalar; chunked so the
            # TRSM and the next rest rotation unblock per-chunk)
            strip = cx["strip"]
            for (cc, cw) in _chunks(rw, 0, FT):
                nc.scalar.copy(
                    strip[:, ds(DW - P + cc, cw)], rp[:, ds(cc, cw)]
                )
                yield

        def evac_pivot(m, i, cx):
            """Pivot -> sblk (+fix) on DVE. dp is complete at round start."""
            dp = cx["dp"]
            dfix = dfix_m[:, i, :] if m == 0 else dfix_s
            nc.vector.tensor_add(cx["sblk"], dp[:, :P], dfix)
            nc.vector.tensor_copy(cx["sb"], cx["sblk"])

        def evac_dstrip(m, i, cx):
            """Diag-region cols [P, dw) -> SBUF strip (scalar). Emitted after
            the refines' first section so sqrt isn't queued behind it."""
            dp, w, dw = cx["dp"], cx["w"], cx["dw"]
            if w > P:
                nc.scalar.copy(cx["strip"][:, : dw - P], dp[:, ds(P, dw - P)])

        def refine_gen(m, i, cx):
            """Pivot-block factor; yields at cross-engine handoffs."""
            sblk, sb = cx["sblk"], cx["sb"]
            dcol = dstore[:, m, ds(i, 1)]
            dummy = rpool.tile([P, P], f32, tag="dummy", name="dummy")
            nc.vector.tensor_mul(dummy, sblk, I128)
            nc.vector.tensor_reduce(dcol, dummy, AX, OP.add)
            rinv = vpool.tile([P, 1], f32, tag="rinv", name="rinv")
            nc.vector.reciprocal(rinv, dcol)
            rcol = vpool.tile([P, 1], f32, tag="rcol", name="rcol")
            nc.scalar.sqrt(rcol, rinv)
            yield
            rt_ps = wpsum.tile([P, FT], f32, tag="w", name="rt_ps")
            nc.tensor.transpose(rt_ps[:1, :P], rcol, I128)
            rrow = vpool.tile([1, P], bf16, tag="rrow", name="rrow")
            nc.vector.tensor_copy(rrow, rt_ps[:1, :P])
            yield
            q_ps = wpsum.tile([P, FT], f32, tag="w", name="q_ps")
            nc.tensor.matmul(q_ps[:, :P], rrow, rrow, start=True, stop=True)
            c1 = rpool.tile([P, P], f32, tag="c1", name="c1")
            nc.vector.tensor_mul(c1, sblk, q_ps[:, :P])
            yield
            x1 = rpool.tile([P, P], bf16, tag="x1", name="x1")
            nc.gpsimd.tensor_mul(x1, c1, STRIU)
            x1t = rpool.tile([P, P], bf16, tag="x1t", name="x1t")
            nc.gpsimd.tensor_mul(x1t, c1, STRIL)
            yield
            x2_ps = wpsum.tile([P, FT], f32, tag="w", name="x2_ps")
            nc.tensor.matmul(x2_ps[:, :P], x1t, x1, start=True, stop=True)
            wser = rpool.tile([P, P], f32, tag="wser", name="wser")
            nc.vector.tensor_sub(wser, x2_ps[:, :P], x1)
            nc.vector.tensor_add(wser, wser, I128)
            wfac = rpool.tile([P, P], bf16, tag="wfac", name="wfac")
            nc.vector.tensor_scalar_mul(wfac, wser, rcol)
            yield
            wt_ps = wpsum.tile([P, FT * 2], bf16, tag="w", name="wt_ps")
            nc.tensor.transpose(wt_ps[:, :P], wfac, I128b)
            wt = rpool.tile([P, P], bf16, tag="wt", name="wt")
            nc.vector.tensor_copy(wt, wt_ps[:, :P])
            yield
            sw_ps = wpsum.tile([P, FT], f32, tag="w", name="sw_ps")
            nc.tensor.matmul(sw_ps[:, :P], sb, wfac, start=True, stop=True)
            swt = rpool.tile([P, P], bf16, tag="swt", name="swt")
            nc.vector.tensor_copy(swt, sw_ps[:, :P])
            yield
            fpi_ps = wpsum.tile([P, FT], f32, tag="w", name="fpi_ps")
            nc.tensor.matmul(fpi_ps[:, :P], wfac, swt, start=True, stop=True)
            ff = rpool.tile([P, P], bf16, tag="ff", name="ff")
            nc.vector.tensor_sub(ff, fpi_ps[:, :P], I128)
            trf = vpool.tile([P, 1], f32, tag="trf", name="trf")
            dummy3 = rpool.tile([P, P], f32, tag="dummy3", name="dummy3")
            nc.gpsimd.tensor_mul(dummy3, ff, I128)
            nc.vector.tensor_reduce(trf, dummy3, AX, OP.add)
            trf2 = vpool.tile([P, 1], f32, tag="trf2", name="trf2")
            dummy4 = rpool.tile([P, P], f32, tag="dummy4", name="dummy4")
            nc.gpsimd.tensor_mul(dummy4, ff, ff)
            nc.vector.tensor_reduce(trf2, dummy4, AX, OP.add)
            yield
            f2_ps = wpsum.tile([P, FT], f32, tag="w", name="f2_ps")
            nc.tensor.matmul(f2_ps[:, :P], ff, ff, start=True, stop=True)
            trf3 = vpool.tile([P, 1], f32, tag="trf3", name="trf3")
            dummy5 = rpool.tile([P, P], f32, tag="dummy5", name="dummy5")
            nc.vector.tensor_mul(dummy5, f2_ps[:, :P], ff)
            nc.vector.tensor_reduce(trf3, dummy5, AX, OP.add)
            f2s = rpool.tile([P, P], bf16, tag="f2s", name="f2s")
            nc.vector.tensor_scalar_mul(f2s, f2_ps[:, :P], 0.375)
            fs = rpool.tile([P, P], bf16, tag="fs", name="fs")
            nc.vector.tensor_scalar_mul(fs, ff, -0.5)
            yield
            wh_ps = wpsum.tile([P, FT], f32, tag="w", name="wh_ps")
            nc.tensor.matmul(wh_ps[:, :P], wt, fs, start=True, stop=False)
            nc.tensor.matmul(wh_ps[:, :P], wt, f2s, start=False, stop=True)
            what = rpool.tile([P, P], bf16, tag="what", name="what")
            nc.vector.tensor_add(what, wh_ps[:, :P], wfac)
            cx["what"] = what
            # logdet trace series accumulation
            t1 = vpool.tile([P, 1], f32, tag="t1", name="t1")
            t2 = vpool.tile([P, 1], f32, tag="t2", name="t2")
            nc.vector.tensor_scalar(
                out=t2, in0=trf2, scalar1=-0.5, scalar2=None, op0=OP.mult
            )
            nc.vector.tensor_add(t1, trf, t2)
            nc.vector.tensor_scalar(
                out=t2, in0=trf3, scalar1=1.0 / 3.0, scalar2=None, op0=OP.mult
            )
            nc.vector.tensor_add(t1, t1, t2)
            nc.vector.tensor_add(acc[:, ds(m, 1)], acc[:, ds(m, 1)], t1)

        def trsm_gen(m, i, cx):
            """U_i = What^T @ strip. Outputs land in the (now free) rest-psum
            banks of this panel plus <=2 work-bank rotations, so the chunk
            matmuls run back-to-back; U (scalar) and -U (vector) evacuations
            pipeline off the PE critical path."""
            T = TDIM[m]
            w = cx["w"]
            c0 = i * P
            last = i == T - 1
            tpr = None
            if w > DW:
                tpr = RPOOL[m].tile([P, RESTW[m]], f32, tag=f"rp{m}", name="tpr")
            rp_off = 0
            for tix, (cc, cw) in enumerate(_chunks(w, c0, FT, diag_first=True)):
                rhs = cx["sb"] if tix == 0 else cx["strip"][:, ds(cc - c0 - P, cw)]
                if tix > 0 and tpr is not None and rp_off + cw <= RESTW[m]:
                    tp = tpr[:, ds(rp_off, cw)]
                    rp_off += FT
                else:
                    tpw = wpsum.tile([P, FT], f32, tag="w", name="tpw")
                    tp = tpw[:, :cw]
                nc.tensor.matmul(tp, cx["what"], rhs, start=True, stop=True)
                nc.scalar.copy(ub[(m, i)][:, ds(cc - c0, cw)], tp)
                if not last and tix > 0:
                    nc.vector.tensor_scalar(
                        out=un[(m, i)][:, ds(cc - c0 - P, cw)],
                        in0=tp,
                        scalar1=-1.0,
                        scalar2=None,
                        op0=OP.mult,
                    )
                yield

        # ---------------- schedule ----------------
        rm_of_round = {r: i for i, r in enumerate(RM)}

        fillers = []  # [key, gen] providing PE filler chunks

        def pump_fillers(k=1, only=None):
            done = 0
            idx = 0
            while idx < len(fillers) and done < k:
                key, g = fillers[idx]
                if only is not None and key not in only:
                    idx += 1
                    continue
                try:
                    next(g)
                    done += 1
                except StopIteration:
                    fillers.pop(idx)

        def dma_bful(kp):
            nc.sync.dma_start(bful[:, 2 * kp, :], bb_r[:, 2 * kp, :])
            nc.sync.dma_start(bful[:, 2 * kp + 1, :], bb_r[:, 2 * kp + 1, :])

        def dma_bsel(kp):
            nc.sync.dma_start(bsel[:, 2 * kp, :], bs_r[:, 2 * kp, :])
            nc.sync.dma_start(bsel[:, 2 * kp + 1, :], bs_r[:, 2 * kp + 1, :])

        cur = {}  # m -> (i, cx) of the panel whose chains are queued/running

        # shared panel 0: diag chain eagerly (streams against its DMAs);
        # the masked panel-0 diag chain becomes round-0 filler. Rest chains
        # are queued at each panel's own processing round (their U_j accum
        # inputs all exist by then).
        cs = new_panel(1, 0)
        for _ in diag_chain(1, 0, cs, 0, dma=dma_bful):
            pass
        cur[1] = (0, cs)
        cm = new_panel(0, 0)
        fillers.append([(0, 0, "d"), diag_chain(0, 0, cm, 0, dma=dma_bsel)])
        cur[0] = (0, cm)

        for r in range(NT):
            work = []  # (m, i, cx) panels processed this round
            mi = rm_of_round.get(r)
            if mi is not None:
                work.append((0, mi, cur[0][1]))
            work.append((1, r, cur[1][1]))

            # diag psums of this round's panels are complete (chains drained
            # and tail accums emitted at the end of the previous round)
            for (m, i, cx) in work:
                evac_pivot(m, i, cx)

            # start the refines' first section (ends at the scalar sqrt)
            # before anything else lands on the scalar queue
            gens = [refine_gen(m, i, cx) for (m, i, cx) in work]
            live = list(gens)
            for g in live:
                next(g)

            # diag-region strip copies (scalar), rest chains (filler), and
            # next-panel diag chains
            nxt = {}
            for (m, i, cx) in work:
                evac_dstrip(m, i, cx)
                fillers.append([(m, i, "r"), rest_chain(m, i, cx)])
            for (m, i, cx) in work:
                if i + 1 < TDIM[m]:
                    nx = new_panel(m, i + 1)
                    fillers.append(
                        [(m, i + 1, "d"), diag_chain(m, i + 1, nx, i)]
                    )
                    nxt[m] = (i + 1, nx)
                    cur[m] = (i + 1, nx)

            # rest of the refines, with rest/diag chains as PE filler
            while live:
                for g in list(live):
                    try:
                        next(g)
                    except StopIteration:
                        live.remove(g)
                    pump_fillers(4)

            # this round's rest chains (incl. strip evac) must be done
            # before their TRSMs read the strip
            pump_fillers(100000, only={(m, i, "r") for (m, i, _) in work})

            # TRSMs, zipped with filler
            tgens = [trsm_gen(m, i, cx) for (m, i, cx) in work]
            live = list(tgens)
            while live:
                for g in list(live):
                    try:
                        next(g)
                    except StopIteration:
                        live.remove(g)
                    pump_fillers(3)

            # close out the next panels' diag psums: drain their diag chains
            # (gram + old accums), then append the just-produced U_i term
            for (m, i, cx) in work:
                if m in nxt:
                    ni, nx = nxt[m]
                    pump_fillers(100000, only={(m, ni, "d")})
                    diag_accum_tail(m, ni, nx, ni)
            # note: masked next-panel rest chains keep pumping in later
            # rounds' refine gaps; they are force-drained before their TRSM
        pump_fillers(100000)  # drain any tail

        # -------- final: batched Ln(d), partition-sum via matmul ------
        lnall = vpool.tile([P, 2, NT], f32, tag="lnall", name="lnall")
        nc.scalar.activation(
            lnall.rearrange("p a b -> p (a b)"),
            dstore.rearrange("p a b -> p (a b)"),
            AF.Ln,
        )
        ln0 = vpool.tile([P, 1], f32, tag="ln0", name="ln0")
        nc.vector.tensor_reduce(ln0, lnall[:, 0, :], AX, OP.add)
        ln1 = vpool.tile([P, 1], f32, tag="ln1", name="ln1")
        nc.vector.tensor_reduce(ln1, lnall[:, 1, :], AX, OP.add)
        accd = vpool.tile([P, 1], f32, tag="accd", name="accd")
        nc.vector.tensor_sub(accd, acc[:, 0:1], acc[:, 1:2])
        nc.vector.tensor_add(accd, accd, ln0)
        nc.vector.tensor_sub(accd, accd, ln1)
        ones = vpool.tile([P, 1], f32, tag="ones", name="ones")
        nc.vector.memset(ones, 1.0)
        r_ps = wpsum.tile([P, FT], f32, tag="w", name="r_ps")
        nc.tensor.matmul(r_ps[:1, :1], accd, ones, start=True, stop=True)
        res = vpool.tile([1, 1], f32, tag="res", name="res")
        nc.vector.tensor_scalar(
            out=res, in0=r_ps[:1, :1], scalar1=1.0, scalar2=OUT_CONST,
            op0=OP.mult, op1=OP.add,
        )
        nc.sync.dma_start(out_d[:, :], res)
        if dbg:
            nc.sync.dma_start(dst_d[:, :], dstore.rearrange("p a b -> p (a b)"))

    nc.finalize()
    return nc


def make_in_maps(x, B):
    """Host-side shard/pack: per-core fp8 inputs."""
    bs, n = x.shape
    k = B.shape[0]
    b8 = np.zeros((N, N), dtype=ml_dtypes.float8_e4m3)
    b8[:k, :] = (B * SCALE).astype(ml_dtypes.float8_e4m3)
    in_maps = []
    for c in range(bs):
        sel = np.nonzero(x[c] == 1)[0]
        ns = len(sel)
        assert ns <= NM, f"sample {c} selects {ns} > {NM} columns"
        bsel = np.zeros((N, NM), dtype=ml_dtypes.float8_e4m3)
        bsel[:, :ns] = b8[:, sel]
        pad = np.zeros((NM, 1), dtype=np.float32)
        pad[ns:] = 1.0
        in_maps.append({"bb": b8, "bs": bsel, "pad": pad})
    return in_maps


def kernel(x, B):
    """Full inputs -> full output. x: [8, 2048] int32, B: [2000, 2048] f32."""
    from concourse.bass_utils import run_bass_kernel_spmd

    bs, n = x.shape
    assert n == N and bs == 8

    if "nc" not in _CACHE:
        _CACHE["nc"] = _build()
    nc = _CACHE["nc"]

    in_maps = make_in_maps(x, B)
    res = run_bass_kernel_spmd(nc, in_maps, core_ids=list(range(bs)))
    out = np.array([r["out"][0, 0] for r in res.results], dtype=np.float32)
    return out



# revision 23
# speedup vs baseline: 1.2900x; 1.2796x over previous
"""Trainium2 Bass kernel v4 for nn_DPP: batched masked-Gram logdet minus
shared normalizer logdet.

out[i] = logdet(G * m_i m_i^T + diag(1-m_i)) - logdet(G + I),  G = B^T B

Sharding: one sample per NeuronCore (B replicated); each core computes its
masked logdet AND the shared logdet; host gathers 8 scalars.

v4 changes vs v3 (~340us -> target <220us): the v3 bottleneck was the
per-round pivot-refine serial chain (~15us x 16 rounds of cross-engine
latency).  v4:
  - TRSM-ready factor is W = D^{-1/2}(I - X1 + X1@X1 + striuh(X1^T X1))
    (2nd-order inverse-Cholesky incl. the A2 term); the critical chain
    ends at wfac.  logdet corrections  tr(F) = sum(W o SW) - P  and
    -tr(F^2)/2  (F = W^T S W - I) run OFF the critical path, as does the
    exact base term -2 sum ln diag(W) (diag extracted from wfac).
  - Diagonal extracted as a ROW via GpSimd partition-reduce so the rsqrt
    row feeds the outer products without any PE transpose; a parallel
    column-form rsqrt feeds the W row-scaling.
  - Masked+shared refines share [P, 2, 128] tiles: one elementwise op
    per step covers both matrices on dual rounds.
  - U panels stored as fp8 pairs (u8 = +U, n8 = -U); accumulations are
    fp8 DoubleRow matmuls over TWO panels at once.  Odd tails use
    single-slot fp8 matmuls (no zero-init needed).
  - All fp8 DR matmuls use 512-col chunks (1024-elem moving AP).
  - TRSM diag-block chunk eliminated (its output is read by nothing).
  - Pipeline: after a panel's first TRSM chunk evacuates, the next
    pivot's single accum-tail matmul fires and the next refine's row
    chain starts under the remaining TRSM + chains.
"""

import numpy as np
import ml_dtypes

P = 128
N = 2048
NM = 1152
NT = N // P         # 16 shared panels
MT = NM // P        # 9 masked panels
NKT = 16            # contraction tiles (2000 rows padded to 2048)
FT = 512
DRT = 512
DW = 512
SCALE = 32.0
S2 = SCALE * SCALE
LN_S2 = float(np.log(S2))
# scale fix + per-refine -1.5P series constant (masked MT refines minus
# shared NT refines)
OUT_CONST = (N - NM) * LN_S2 + 1.5 * P * (NT - MT)

RM = [1, 3, 5, 7, 9, 11, 13, 14, 15]  # masked panel i -> round RM[i]

_CACHE = {}


def _chunks(width, base, step):
    out = []
    c = base
    end = base + width
    while c < end:
        w = min(step, end - c)
        out.append((c, w))
        c += w
    return out


def _build():
    import os
    import concourse.bass as bass
    import concourse.bacc as bacc
    import concourse.mybir as mybir
    from concourse.bass import ds
    from concourse.masks import (
        make_identity,
        make_upper_triangular,
        make_lower_triangular,
    )
    from concourse.tile import TileContext
    from contextlib import ExitStack

    f32 = mybir.dt.float32
    bf16 = mybir.dt.bfloat16
    fp8 = mybir.dt.float8e4
    AF = mybir.ActivationFunctionType
    OP = mybir.AluOpType
    DR = mybir.MatmulPerfMode.DoubleRow
    PSUM = bass.MemorySpace.PSUM
    AX = mybir.AxisListType.X
    AXY = mybir.AxisListType.XY
    AC = mybir.AxisListType.C

    nc = bacc.Bacc()
    bb = nc.dram_tensor("bb", [N, N], fp8, kind="ExternalInput")
    bs_d = nc.dram_tensor("bs", [N, NM], fp8, kind="ExternalInput")
    pad_d = nc.dram_tensor("pad", [NM, 1], f32, kind="ExternalInput")
    out_d = nc.dram_tensor("out", [1, 1], f32, kind="ExternalOutput")

    TDIM = [MT, NT]
    DIMW = [NM, N]
    NPAIR = [(MT + 1) // 2, (NT + 1) // 2]
    RESTW = [NM - DW, N - DW]     # rest-chain widths
    BANKW = [1024, 1536]          # rest psum capacity (banks)

    with TileContext(nc) as tc, ExitStack() as stack:
        consts = stack.enter_context(tc.tile_pool(name="consts", bufs=1))
        I128 = consts.tile([P, P], f32, tag="i128")
        make_identity(nc, I128)
        I2f = consts.tile([P, 2, P], f32, tag="i2f")
        nc.vector.tensor_copy(I2f[:, 0, :], I128)
        nc.vector.tensor_copy(I2f[:, 1, :], I128)
        STRIU = consts.tile([P, 2, P], f32, tag="striu")
        make_upper_triangular(nc, STRIU[:, 0, :], val=1.0, diag=False)
        nc.vector.tensor_copy(STRIU[:, 1, :], STRIU[:, 0, :])
        STRIL = consts.tile([P, 2, P], f32, tag="stril")
        make_lower_triangular(nc, STRIL[:, 0, :], val=1.0, diag=False)
        nc.vector.tensor_copy(STRIL[:, 1, :], STRIL[:, 0, :])
        STRIUH = consts.tile([P, 2, P], f32, tag="striuh")
        nc.vector.tensor_scalar(
            out=STRIUH[:, 0, :], in0=I128, scalar1=0.5, scalar2=None,
            op0=OP.mult,
        )
        nc.vector.tensor_add(STRIUH[:, 0, :], STRIUH[:, 0, :], STRIU[:, 0, :])
        nc.vector.tensor_copy(STRIUH[:, 1, :], STRIUH[:, 0, :])

        padc = consts.tile([P, MT], f32, tag="padc")
        nc.sync.dma_start(padc, pad_d.rearrange("(t p) one -> p (t one)", p=P))

        accA = consts.tile([P, 2], f32, tag="accA")
        nc.vector.memset(accA, 0.0)
        accB = consts.tile([P, 2], f32, tag="accB")
        nc.vector.memset(accB, 0.0)
        dstore = consts.tile([P, NT, 2], f32, tag="dstore")
        nc.vector.memset(dstore, 1.0)

        dfix_m = consts.tile([P, MT, P], f32, tag="dfix_m")
        pscl = consts.tile([P, MT], f32, tag="pscl")
        nc.vector.tensor_scalar(
            out=pscl, in0=padc, scalar1=S2, scalar2=None, op0=OP.mult
        )
        for i in range(MT):
            nc.vector.tensor_scalar_mul(dfix_m[:, i, :], I128, pscl[:, ds(i, 1)])
        dfix_s = consts.tile([P, P], f32, tag="dfix_s")
        nc.vector.tensor_scalar(
            out=dfix_s, in0=I128, scalar1=S2, scalar2=None, op0=OP.mult
        )

        bsel = consts.tile([P, NKT, NM], fp8, tag="bsel")
        bful = consts.tile([P, NKT, N], fp8, tag="bful")
        bs_r = bs_d.rearrange("(t p) n -> p t n", p=P)
        bb_r = bb.rearrange("(t p) n -> p t n", p=P)

        FP8ACC = os.environ.get("KV4_NOFP8", "0") != "1"
        SAFE = os.environ.get("KV4_SAFE", "0") == "1"
        NROUNDS = int(os.environ.get("KV4_ROUNDS", str(NT)))
        edt = fp8 if FP8ACC else bf16
        # pair tile p only holds cols [2pP, DIM) (earlier cols never read)
        u8 = {}
        n8 = {}
        for m in range(2):
            for pr in range(NPAIR[m]):
                wp = DIMW[m] - 2 * pr * P
                u8[(m, pr)] = consts.tile(
                    [P, 2, wp], edt, tag=f"u8_{m}_{pr}", name=f"u8_{m}_{pr}"
                )
                n8[(m, pr)] = consts.tile(
                    [P, 2, wp], edt, tag=f"n8_{m}_{pr}", name=f"n8_{m}_{pr}"
                )

        ddiagp = stack.enter_context(tc.tile_pool(name="ddiag", bufs=1, space=PSUM))
        mrestp = stack.enter_context(tc.tile_pool(name="mrest", bufs=1, space=PSUM))
        srestp = stack.enter_context(tc.tile_pool(name="srest", bufs=1, space=PSUM))
        wpsump = stack.enter_context(tc.tile_pool(name="wpsum", bufs=1, space=PSUM))
        spool = stack.enter_context(tc.tile_pool(name="spool", bufs=2))
        rpool = stack.enter_context(tc.tile_pool(name="rpool", bufs=2))
        vpool = stack.enter_context(tc.tile_pool(name="vpool", bufs=2))

        ddiag = ddiagp.tile([P, 2, DW], f32, tag="ddiag", name="ddiag")
        wpsum = wpsump.tile([P, FT], f32, tag="wpsum", name="wpsum")
        RPOOL = [mrestp, srestp]

        X = [bsel, bful]

        def new_panel(m, i):
            T = TDIM[m]
            w = (T - i) * P
            cx = {"w": w, "dw": min(DW, w), "dp": ddiag[:, m, :],
                  "rp": None, "strip": None}
            if w > DW:
                cx["rp"] = RPOOL[m].tile([P, BANKW[m]], f32, tag=f"rp{m}",
                                         name="rp")
            if w > P:
                cx["strip"] = spool.tile([P, w - P], bf16, tag=f"strip{m}",
                                         name="strip")
            return cx

        def dr_gram(dst, m, kp, c0, cc, cw):
            nc.tensor.matmul(
                dst,
                X[m][:, 2 * kp : 2 * kp + 2, ds(c0, P)],
                X[m][:, 2 * kp : 2 * kp + 2, ds(cc, cw)],
                start=(kp == 0), stop=False, perf_mode=DR,
                skip_group_check=True,
            )

        def dr_acc(dst, m, pr, i, cc, cw, stop):
            o = 2 * pr * P
            if not FP8ACC:
                nc.tensor.matmul(
                    dst, n8[(m, pr)][:, 0, ds(i * P - o, P)],
                    u8[(m, pr)][:, 0, ds(cc - o, cw)],
                    start=False, stop=False, skip_group_check=True,
                )
                nc.tensor.matmul(
                    dst, n8[(m, pr)][:, 1, ds(i * P - o, P)],
                    u8[(m, pr)][:, 1, ds(cc - o, cw)],
                    start=False, stop=stop, skip_group_check=True,
                )
                return
            nc.tensor.matmul(
                dst,
                n8[(m, pr)][:, :, ds(i * P - o, P)],
                u8[(m, pr)][:, :, ds(cc - o, cw)],
                start=False, stop=stop, perf_mode=DR, skip_group_check=True,
            )

        def single_acc(dst, m, j, i, cc, cw, stop):
            pr, sl = j // 2, j % 2
            o = 2 * pr * P
            nc.tensor.matmul(
                dst,
                n8[(m, pr)][:, sl, ds(i * P - o, P)],
                u8[(m, pr)][:, sl, ds(cc - o, cw)],
                start=False, stop=stop, skip_group_check=True,
            )

        def diag_chain(m, i, cx, dma=None):
            """Gram + queue-time-available accums (panels <= i-2) for cols
            [iP, iP+dw).  The last term arrives via diag_tail."""
            dp, dw = cx["dp"], cx["dw"]
            c0 = i * P
            for kp in range(NKT // 2):
                if dma is not None:
                    dma(kp)
                dr_gram(dp[:, :dw], m, kp, c0, c0, dw)
                yield
            npr = (i - 1) // 2 if i % 2 == 1 else max(i - 2, 0) // 2
            for pr in range(npr):
                dr_acc(dp[:, :dw], m, pr, i, c0, dw, False)
                yield

        def diag_tail(m, i, cx):
            dp, dw = cx["dp"], cx["dw"]
            c0 = i * P
            if i % 2 == 0 and i >= 2:
                dr_acc(dp[:, :dw], m, (i - 2) // 2, i, c0, dw, True)
            elif i % 2 == 1:
                single_acc(dp[:, :dw], m, i - 1, i, c0, dw, True)

        def rest_chain(m, i, cx):
            """Gram + full accum (panels <= i-1, all available when queued
            at the end of round i-1) for cols [c0+DW, c0+w), then strip
            evac.  Pumped as filler during round i, drained pre-TRSM."""
            w, rp = cx["w"], cx["rp"]
            if rp is None:
                return
            c0 = i * P
            rw = w - DW
            for kp in range(NKT // 2):
                for (cc, cw) in _chunks(rw, c0 + DW, DRT):
                    dr_gram(rp[:, ds(cc - c0 - DW, cw)], m, kp, c0, cc, cw)
                    yield
            pairs = list(range(i // 2))
            single_j = i - 1 if i % 2 == 1 else None
            nacc = len(pairs) + (1 if single_j is not None else 0)
            k = 0
            for pr in pairs:
                k += 1
                for (cc, cw) in _chunks(rw, c0 + DW, DRT):
                    dr_acc(rp[:, ds(cc - c0 - DW, cw)], m, pr, i, cc, cw,
                           k == nacc)
                    yield
            if single_j is not None:
                k += 1
                for (cc, cw) in _chunks(rw, c0 + DW, DRT):
                    single_acc(rp[:, ds(cc - c0 - DW, cw)], m, single_j, i,
                               cc, cw, k == nacc)
                    yield
            strip = cx["strip"]
            for (cc, cw) in _chunks(rw, 0, FT):
                nc.scalar.copy(strip[:, ds(DW - P + cc, cw)], rp[:, ds(cc, cw)])
                yield

        def evac_dstrip(m, i, cx):
            dp, w, dw = cx["dp"], cx["w"], cx["dw"]
            if w > P:
                nc.scalar.copy(cx["strip"][:, : dw - P], dp[:, ds(P, dw - P)])

        # ---------------- refine ----------------
        def rtile(pool, shape, dt, tag):
            return pool.tile(shape, dt, tag=tag, name=tag)

        def refine_gen(rnd, A):
            duo = len(A) == 2
            lo = 0 if duo else A[0][0]
            nsl = 2 if duo else 1

            def sl(t):
                return t[:, lo : lo + nsl, :]

            sblk2 = rtile(rpool, [P, 2, P], f32, "sblk2")
            c1s = rtile(rpool, [P, 2, P], f32, "c1s")
            x1 = rtile(rpool, [P, 2, P], bf16, "x1")
            x1t = rtile(rpool, [P, 2, P], bf16, "x1t")
            x1mi = rtile(rpool, [P, 2, P], f32, "x1mi")
            a2c = rtile(rpool, [P, 2, P], f32, "a2c")
            x1ms = rtile(rpool, [P, 2, P], f32, "x1ms")
            wfac = rtile(rpool, [P, 2, P], bf16, "wfac")
            sb2 = rtile(rpool, [P, 2, P], bf16, "sb2")
            swt = rtile(rpool, [P, 2, P], bf16, "swt")
            fcop = rtile(rpool, [P, 2, P], f32, "fcop")
            scr = rtile(rpool, [P, 2, P], f32, "scr")
            rrow = rtile(vpool, [1, 2, P], bf16, "rrow")
            dcol = rtile(vpool, [P, 2], f32, "dcol")
            rtmp = rtile(vpool, [P, 2], f32, "rtmp")
            for (m, i, cx) in A:
                cx["wfac"] = wfac[:, m, :]

            # stage 0: pivot copy; fused diag-extract+fix; rsqrt column
            if duo:
                nc.vector.tensor_copy(sblk2, ddiag[:, :, :P])
            else:
                nc.vector.tensor_copy(sl(sblk2), ddiag[:, lo, :P])
            if SAFE:
                nc.vector.tensor_mul(sl(scr), sl(sblk2), sl(I2f))
                nc.vector.tensor_reduce(
                    dcol[:, lo : lo + nsl], sl(scr), AX, OP.add
                )
                for (m, i, cx) in A:
                    if m == 0:
                        nc.vector.tensor_add(
                            dcol[:, 0:1], dcol[:, 0:1], pscl[:, ds(i, 1)]
                        )
                    else:
                        nc.vector.tensor_scalar(
                            out=dcol[:, 1:2], in0=dcol[:, 1:2], scalar1=1.0,
                            scalar2=float(S2), op0=OP.mult, op1=OP.add,
                        )
            else:
                for (m, i, cx) in A:
                    fix = pscl[:, ds(i, 1)] if m == 0 else float(S2)
                    nc.vector.tensor_tensor_reduce(
                        out=scr[:, m, :], in0=sblk2[:, m, :], in1=I2f[:, m, :],
                        scale=1.0, scalar=fix, op0=OP.mult, op1=OP.add,
                        accum_out=dcol[:, ds(m, 1)],
                    )
            nc.vector.reciprocal(
                dcol[:, lo : lo + nsl], dcol[:, lo : lo + nsl]
            )
            nc.scalar.sqrt(rtmp[:, lo : lo + nsl], dcol[:, lo : lo + nsl])
            yield

            # stage 1: transposes -> rrow; outer products -> q; c1
            for (m, i, cx) in A:
                nc.tensor.transpose(
                    wpsum[:1, ds(256 + m * P, P)], rtmp[:, ds(m, 1)], I128
                )
            nc.vector.tensor_copy(
                rrow[:, lo : lo + nsl, :],
                wpsum[:1, ds(256 + lo * P, nsl * P)].rearrange(
                    "p (s q) -> p s q", q=P),
            )
            for (m, i, cx) in A:
                nc.tensor.matmul(
                    wpsum[:, ds(m * P, P)], rrow[:, m, :], rrow[:, m, :],
                    start=True, stop=True, skip_group_check=True,
                )
            nc.vector.tensor_mul(
                sl(c1s), sl(sblk2),
                wpsum[:, ds(lo * P, nsl * P)].rearrange(
                    "p (s q) -> p s q", q=P),
            )
            yield

            # stage 2: triangular masks (x1t on DVE, x1/x1mi on GpSimd)
            nc.gpsimd.tensor_mul(sl(x1), sl(c1s), sl(STRIU))
            nc.vector.tensor_mul(sl(x1t), sl(c1s), sl(STRIL))
            if SAFE:
                nc.vector.tensor_sub(sl(x1mi), sl(x1), sl(I2f))
            else:
                nc.gpsimd.tensor_sub(sl(x1mi), sl(x1), sl(I2f))
            yield

            # stage 3: x2 = X1@X1 (lo bank half), xtx = X1^T X1 (hi half)
            for (m, i, cx) in A:
                nc.tensor.matmul(
                    wpsum[:, ds(m * P, P)], x1t[:, m, :], x1[:, m, :],
                    start=True, stop=True, skip_group_check=True,
                )
            for (m, i, cx) in A:
                nc.tensor.matmul(
                    wpsum[:, ds(256 + m * P, P)], x1[:, m, :], x1[:, m, :],
                    start=True, stop=True, skip_group_check=True,
                )
            yield

            # stage 4 (all DVE): wfac = (x2 - (x1 - I - a2c)) o r
            nc.vector.tensor_mul(
                sl(a2c),
                wpsum[:, ds(256 + lo * P, nsl * P)].rearrange(
                    "p (s q) -> p s q", q=P),
                sl(STRIUH),
            )
            nc.vector.tensor_sub(sl(x1mi), sl(x1mi), sl(a2c))
            for (m, i, cx) in A:
                nc.vector.tensor_scalar_mul(
                    x1ms[:, m, :], x1mi[:, m, :], rtmp[:, ds(m, 1)]
                )
            if SAFE:
                nc.vector.tensor_sub(
                    sl(scr),
                    wpsum[:, ds(lo * P, nsl * P)].rearrange(
                        "p (s q) -> p s q", q=P),
                    sl(x1mi),
                )
                for (m, i, cx) in A:
                    nc.vector.tensor_scalar_mul(
                        wfac[:, m, :], scr[:, m, :], rtmp[:, ds(m, 1)]
                    )
            else:
                for (m, i, cx) in A:
                    nc.vector.scalar_tensor_tensor(
                        out=wfac[:, m, :],
                        in0=wpsum[:, ds(m * P, P)],
                        scalar=rtmp[:, ds(m, 1)],
                        in1=x1ms[:, m, :],
                        op0=OP.mult, op1=OP.subtract,
                    )
            yield

            # stage 5 (off-path): sw matmuls; tr(F) and diag(W) accums
            for (m, i, cx) in A:
                dfix = dfix_m[:, i, :] if m == 0 else dfix_s
                nc.vector.tensor_add(sb2[:, m, :], sblk2[:, m, :], dfix)
            for (m, i, cx) in A:
                nc.tensor.matmul(
                    wpsum[:, ds(m * P, P)], sb2[:, m, :], wfac[:, m, :],
                    start=True, stop=True, skip_group_check=True,
                )
            nc.vector.tensor_copy(
                sl(swt),
                wpsum[:, ds(lo * P, nsl * P)].rearrange("p (s q) -> p s q", q=P),
            )
            if SAFE:
                nc.vector.tensor_mul(
                    sl(scr), sl(wfac),
                    wpsum[:, ds(lo * P, nsl * P)].rearrange(
                        "p (s q) -> p s q", q=P),
                )
                nc.vector.tensor_reduce(
                    dcol[:, lo : lo + nsl], sl(scr), AX, OP.add
                )
                nc.vector.tensor_add(
                    accA[:, lo : lo + nsl], accA[:, lo : lo + nsl],
                    dcol[:, lo : lo + nsl],
                )
                nc.vector.tensor_mul(sl(scr), sl(wfac), sl(I2f))
                nc.vector.tensor_reduce(
                    dstore[:, rnd, lo : lo + nsl], sl(scr), AX, OP.add
                )
            else:
                for (m, i, cx) in A:
                    nc.vector.tensor_tensor_reduce(
                        out=scr[:, m, :], in0=wfac[:, m, :],
                        in1=wpsum[:, ds(m * P, P)], scale=1.0,
                        scalar=accA[:, ds(m, 1)], op0=OP.mult, op1=OP.add,
                        accum_out=accA[:, ds(m, 1)],
                    )
                for (m, i, cx) in A:
                    nc.vector.tensor_tensor_reduce(
                        out=scr[:, m, :], in0=wfac[:, m, :], in1=I2f[:, m, :],
                        scale=1.0, scalar=0.0, op0=OP.mult, op1=OP.add,
                        accum_out=dstore[:, rnd, ds(m, 1)],
                    )
            yield

            # stage 6 (off-path): fpi matmuls; tr(F^2) accum
            for (m, i, cx) in A:
                nc.tensor.matmul(
                    wpsum[:, ds(256 + m * P, P)], wfac[:, m, :], swt[:, m, :],
                    start=True, stop=True, skip_group_check=True,
                )
            nc.vector.tensor_copy(
                sl(fcop),
                wpsum[:, ds(256 + lo * P, nsl * P)].rearrange(
                    "p (s q) -> p s q", q=P),
            )
            if SAFE:
                nc.vector.tensor_mul(sl(scr), sl(fcop), sl(fcop))
                nc.vector.tensor_reduce(
                    dcol[:, lo : lo + nsl], sl(scr), AX, OP.add
                )
                nc.vector.tensor_add(
                    accB[:, lo : lo + nsl], accB[:, lo : lo + nsl],
                    dcol[:, lo : lo + nsl],
                )
            else:
                for (m, i, cx) in A:
                    nc.vector.tensor_tensor_reduce(
                        out=scr[:, m, :],
                        in0=fcop[:, m, :],
                        in1=fcop[:, m, :],
                        scale=1.0, scalar=accB[:, ds(m, 1)],
                        op0=OP.mult, op1=OP.add,
                        accum_out=accB[:, ds(m, 1)],
                    )

        def trsm_gen(m, i, cx):
            """U_i[:, P:] = wfac^T @ strip (no diag chunk).  Chunks land in
            this panel's rest banks; shared overflow chunk in wpsum."""
            w = cx["w"]
            if w <= P:
                return
            c0 = i * P
            rp = cx["rp"]
            pr, slot = i // 2, i % 2
            rp_off = 0
            for (cc, cw) in _chunks(w - P, c0 + P, FT):
                if rp is not None and rp_off + cw <= BANKW[m]:
                    tp = rp[:, ds(rp_off, cw)]
                    rp_off += cw
                else:
                    tp = wpsum[:, ds(128, cw)]
                nc.tensor.matmul(
                    tp, cx["wfac"], cx["strip"][:, ds(cc - c0 - P, cw)],
                    start=True, stop=True, skip_group_check=True,
                )
                o = 2 * pr * P
                nc.scalar.copy(u8[(m, pr)][:, slot, ds(cc - o, cw)], tp)
                nc.vector.tensor_scalar(
                    out=n8[(m, pr)][:, slot, ds(cc - o, cw)], in0=tp,
                    scalar1=-1.0, scalar2=None, op0=OP.mult,
                )
                yield

        # ---------------- schedule ----------------
        rm_of_round = {r: i for i, r in enumerate(RM)}
        fillers = []

        def pump_fillers(k=1, only=None):
            done = 0
            idx = 0
            while idx < len(fillers) and done < k:
                key, g = fillers[idx]
                if only is not None and key not in only:
                    idx += 1
                    continue
                try:
                    next(g)
                    done += 1
                except StopIteration:
                    fillers.pop(idx)

        def dma_bful(kp):
            nc.sync.dma_start(bful[:, 2 * kp, :], bb_r[:, 2 * kp, :])
            nc.sync.dma_start(bful[:, 2 * kp + 1, :], bb_r[:, 2 * kp + 1, :])

        def dma_bsel(kp):
            nc.sync.dma_start(bsel[:, 2 * kp, :], bs_r[:, 2 * kp, :])
            nc.sync.dma_start(bsel[:, 2 * kp + 1, :], bs_r[:, 2 * kp + 1, :])

        # bootstrap
        cs = new_panel(1, 0)
        for _ in diag_chain(1, 0, cs, dma=dma_bful):
            pass
        cm = new_panel(0, 0)
        fillers.append([(0, 0, "d"), diag_chain(0, 0, cm, dma=dma_bsel)])
        cur0 = {0: (0, cm)}
        fillers.append([(1, 0, "r"), rest_chain(1, 0, cs)])
        evac_dstrip(1, 0, cs)
        curA = [(1, 0, cs)]
        ref = refine_gen(0, curA)
        next(ref)   # stage 0
        pump_fillers(4)
        next(ref)   # stage 1
        pump_fillers(4)

        for r in range(NROUNDS):
            # refine stages 2..4 with filler between
            for _ in range(3):
                next(ref)
                pump_fillers(3)
            # this round's rest chains must be fully emitted before TRSM
            pump_fillers(100000, only={(m, i, "r") for (m, i, cx) in curA})

            # next round's panels: create + queue diag chains
            nxtA = []
            r1 = r + 1
            if r1 < NT:
                mi1 = rm_of_round.get(r1)
                if mi1 is not None:
                    if mi1 == 0:
                        nx = cur0[0][1]
                        pump_fillers(100000, only={(0, 0, "d")})
                    else:
                        nx = new_panel(0, mi1)
                        fillers.append([(0, mi1, "d"),
                                        diag_chain(0, mi1, nx)])
                    nxtA.append((0, mi1, nx))
                ns_ = new_panel(1, r1)
                fillers.append([(1, r1, "d"), diag_chain(1, r1, ns_)])
                nxtA.append((1, r1, ns_))

            # TRSM first chunks (rel [P, P+512)) + evacs
            tgens = []
            for (m, i, cx) in curA:
                g = trsm_gen(m, i, cx)
                try:
                    next(g)
                    tgens.append(g)
                except StopIteration:
                    pass
            # drain next diag chains, then the closing tails
            for (m, ni, nx) in nxtA:
                pump_fillers(100000, only={(m, ni, "d")})
                diag_tail(m, ni, nx)

            nref = None
            if nxtA:
                nref = refine_gen(r1, nxtA)
                next(nref)  # stage 0 (row chain; runs under TRSM)

            # remaining TRSM chunks + current refine stages 5,6 + fillers
            for g in tgens:
                for _ in g:
                    pump_fillers(2)
            for _ in ref:
                pump_fillers(2)

            if nxtA:
                for (m, ni, nx) in nxtA:
                    fillers.append([(m, ni, "r"), rest_chain(m, ni, nx)])
                    evac_dstrip(m, ni, nx)
                next(nref)  # stage 1
                pump_fillers(2)
                ref = nref
                curA = nxtA
        if NROUNDS < NT:
            for _ in ref:
                pump_fillers(2)
        pump_fillers(100000)

        # -------- final: batched Ln, combine, partition-sum --------
        lnall = vpool.tile([P, NT, 2], f32, tag="lnall", name="lnall")
        nc.scalar.activation(
            lnall.rearrange("p a b -> p (a b)"),
            dstore.rearrange("p a b -> p (a b)"),
            AF.Ln,
        )
        ln0 = vpool.tile([P, 1], f32, tag="ln0", name="ln0")
        nc.vector.tensor_reduce(ln0, lnall[:, :, 0:1], AXY, OP.add)
        ln1 = vpool.tile([P, 1], f32, tag="ln1", name="ln1")
        nc.vector.tensor_reduce(ln1, lnall[:, :, 1:2], AXY, OP.add)
        # acc = -2(ln0 - ln1) + 2(accA0 - accA1) - 0.5(accB0 - accB1)
        accd = vpool.tile([P, 1], f32, tag="accd", name="accd")
        t0 = vpool.tile([P, 1], f32, tag="t0", name="t0")
        nc.vector.tensor_sub(accd, ln1, ln0)
        nc.vector.tensor_scalar(
            out=accd, in0=accd, scalar1=2.0, scalar2=None, op0=OP.mult
        )
        nc.vector.tensor_sub(t0, accA[:, 0:1], accA[:, 1:2])
        nc.vector.tensor_scalar(
            out=t0, in0=t0, scalar1=2.0, scalar2=None, op0=OP.mult
        )
        nc.vector.tensor_add(accd, accd, t0)
        nc.vector.tensor_sub(t0, accB[:, 0:1], accB[:, 1:2])
        nc.vector.tensor_scalar(
            out=t0, in0=t0, scalar1=-0.5, scalar2=None, op0=OP.mult
        )
        nc.vector.tensor_add(accd, accd, t0)
        ones = vpool.tile([P, 1], f32, tag="ones", name="ones")
        nc.vector.memset(ones, 1.0)
        nc.tensor.matmul(wpsum[:1, :1], accd, ones, start=True, stop=True,
                         skip_group_check=True)
        res = vpool.tile([1, 1], f32, tag="res", name="res")
        nc.vector.tensor_scalar(
            out=res, in0=wpsum[:1, :1], scalar1=1.0, scalar2=OUT_CONST,
            op0=OP.mult, op1=OP.add,
        )
        nc.sync.dma_start(out_d[:, :], res)

    nc.finalize()
    return nc


def make_in_maps(x, B):
    bs, n = x.shape
    k = B.shape[0]
    b8 = np.zeros((N, N), dtype=ml_dtypes.float8_e4m3)
    b8[:k, :] = (B * SCALE).astype(ml_dtypes.float8_e4m3)
    rm_of_round = {r: i for i, r in enumerate(RM)}
    in_maps = []
    for c in range(bs):
        sel = np.nonzero(x[c] == 1)[0]
        ns = len(sel)
        assert ns <= NM, f"sample {c} selects {ns} > {NM} columns"
        bsel = np.zeros((N, NM), dtype=ml_dtypes.float8_e4m3)
        bsel[:, :ns] = b8[:, sel]
        pad = np.zeros((NM, 1), dtype=np.float32)
        pad[ns:] = 1.0
        in_maps.append({"bb": b8, "bs": bsel, "pad": pad})
    return in_maps


def kernel(x, B):
    """Full inputs -> full output. x: [8, 2048] int32, B: [2000, 2048] f32."""
    from concourse.bass_utils import run_bass_kernel_spmd

    bs, n = x.shape
    assert n == N and bs == 8

    if "nc" not in _CACHE:
        _CACHE["nc"] = _build()
    nc = _CACHE["nc"]

    in_maps = make_in_maps(x, B)
    res = run_bass_kernel_spmd(nc, in_maps, core_ids=list(range(bs)))
    out = np.array([r["out"][0, 0] for r in res.results], dtype=np.float32)
    return out
